# revision 8
# baseline (speedup 1.0000x reference)
"""Trainium2 Bass kernel for nn_DiffusionLayer (ADI diffusion, 10 steps).

Mathematical collapse: every sweep of the ADI scheme is a fixed tridiagonal
solve shared by all rows (the coefficients depend only on the size-128
parameter vectors and the time index, never on u). Each x-sweep is a right
multiplication V <- V @ Mx^T and each y-sweep a left multiplication
V <- My @ V of the 128x128 image V. Left and right multiplications commute,
so the whole 30-sweep scheme is

    V_out = L @ V @ R,   L = My_10 ... My_1,   R = Mx_1^T Mx_2^T ... Mx_20^T

with L, R computed on host in float64 (including the reference's EPS
perturbations of the Thomas recurrences).

Device pipeline (fp16 wire + fp16 matmuls, fp32 PSUM accumulate):

    P1 = matmul(lhsT=V,  rhs=L^T)          = (L V)^T    [w,  h']  (per image)
    W  = matmul(lhsT=R,  rhs=P1 quad-wide) = ((L V) R)^T [w', h']  (N=512)

fp16 halves HBM traffic vs fp32 (the memory roofline) and runs the PE at
1 cycle/row instead of fp32's 4. The second matmul keeps R stationary and
streams 4 images at once. Output leaves transposed; the host untransposes.
Host packs inputs as [h, b, w] (and reads outputs as [w, b, h]) so every
DMA is >=2KB contiguous per partition.

Sharding: pure data parallelism, 2048 images -> 256 per core across 8 cores.
"""

import numpy as np

import concourse.mybir as mybir
import concourse.tile as tile
from concourse import bacc
from concourse.bass_utils import run_bass_kernel_spmd

N_CORES = 8
BATCH = 2048
S = 128
PER_CORE = BATCH // N_CORES  # 256

SIZE, DT, DX, DY, NUM_STEPS, EPS = 128, 0.01, 1.0, 1.0, 10, 1e-6

GIMG = 16            # images per DMA group (512 KB per transfer at fp16)
NGRP = PER_CORE // GIMG
OCT = 8              # images per PSUM tile (2 banks) / per PSUM->SBUF copy


# ----------------------------------------------------------------- host math
def _smooth3(v):
    vp = np.pad(v, (1, 1), mode="edge")
    return (vp[:-2] + vp[1:-1] + vp[2:]) / 3.0


def _thomas_matrix(a, b, c):
    """Matrix M of the reference thomas() linear map d -> x (includes EPS)."""
    n = len(b)
    dn = np.empty(n)
    cs = np.empty(n)
    dn[0] = b[0] + EPS
    cs[0] = c[0] / dn[0]
    for i in range(1, n):
        dn[i] = b[i] - a[i] * cs[i - 1] + EPS
        cs[i] = c[i] / dn[i]
    ds = np.empty((n, n))
    ds[0] = np.eye(n)[0] / dn[0]
    eye = np.eye(n)
    for i in range(1, n):
        ds[i] = (eye[i] - a[i] * ds[i - 1]) / dn[i]
    x = np.empty((n, n))
    x[n - 1] = ds[n - 1]
    for i in range(n - 2, -1, -1):
        x[i] = ds[i] - cs[i] * x[i + 1]
    return x


def _sweep_matrix(vec, dt, dh):
    coeff = _smooth3(vec) * dt / dh**2
    a = -coeff
    c = -coeff
    b = 1.0 + 2.0 * coeff
    b = b.copy()
    b[0] = 1.0 + coeff[0]
    b[-1] = 1.0 + coeff[-1]
    return _thomas_matrix(a, b, c)


def _coef(base, lin, quad, t):
    return np.clip(base + lin * t + quad * t * t, EPS, None)


def _build_lr(abx, atx, aqx, bby, bty, bqy):
    """L (y-operator product) and R (x-operator product) in float64."""
    L = np.eye(SIZE)
    R = np.eye(SIZE)
    t = 0.0
    for _ in range(NUM_STEPS):
        Mx = _sweep_matrix(_coef(abx, atx, aqx, t), DT / 2, DX)
        R = R @ Mx.T
        t += DT / 2
        My = _sweep_matrix(_coef(bby, bty, bqy, t), DT, DY)
        L = My @ L
        t += DT / 2
        Mx = _sweep_matrix(_coef(abx, atx, aqx, t), DT / 2, DX)
        R = R @ Mx.T
    return L, R


# ------------------------------------------------------------- device kernel
_NC_CACHE = {}


def _build_nc():
    if "nc" in _NC_CACHE:
        return _NC_CACHE["nc"]
    f16 = mybir.dt.float16
    f32 = mybir.dt.float32
    nc = bacc.Bacc(None)
    # u is packed [h, b, w]; out leaves as [w', b, h'] (host untransposes)
    u_in = nc.dram_tensor("u", [S, PER_CORE, S], f16, kind="ExternalInput")
    lt_in = nc.dram_tensor("lt", [S, S], f16, kind="ExternalInput")
    r_in = nc.dram_tensor("rm", [S, S], f16, kind="ExternalInput")
    u_out = nc.dram_tensor("out", [S, PER_CORE, S], f16, kind="ExternalOutput")

    with tile.TileContext(nc) as tc:
        with (
            tc.tile_pool(name="mats", bufs=1) as mats,
            tc.tile_pool(name="inp", bufs=NGRP) as inp,
            tc.tile_pool(name="outp", bufs=2 * NGRP) as outp,
            tc.tile_pool(name="mid", bufs=4) as mid,
            tc.tile_pool(name="ps1", bufs=2, space="PSUM") as ps1,
            tc.tile_pool(name="ps2", bufs=2, space="PSUM") as ps2,
        ):
            lt_s = mats.tile([S, S], f16)
            r_s = mats.tile([S, S], f16)
            # matrices ride the scalar HWDGE ring so the sync ring starts
            # streaming input immediately
            nc.scalar.dma_start(out=lt_s[:], in_=lt_in[:])
            nc.scalar.dma_start(out=r_s[:], in_=r_in[:])

            # ACT copies run ~1114 ns vs DVE's ~1223 ns (FD=1024, PSUM src),
            # so ACT takes 17 of every 32 copies; consecutive copies
            # alternate engines so each oct's mid/out pair is split
            copy_idx = [0]
            act_share = [0]

            def copy(dst, src):
                i = copy_idx[0]
                copy_idx[0] += 1
                want_act = ((i + 1) * 17) // 32 != (i * 17) // 32
                if want_act:
                    nc.scalar.activation(dst, src, mybir.ActivationFunctionType.Copy)
                    act_share[0] += 1
                else:
                    nc.vector.tensor_copy(dst, src)

            for g in range(NGRP):
                g0 = g * GIMG
                in_t = inp.tile([S, GIMG, S], f16)
                if g == 0:
                    # fine-grained first loads so the PE starts sooner
                    for c in range(0, GIMG, 4):
                        nc.sync.dma_start(
                            out=in_t[:, c : c + 4, :],
                            in_=u_in[:, g0 + c : g0 + c + 4, :],
                        )
                else:
                    nc.sync.dma_start(out=in_t[:], in_=u_in[:, g0 : g0 + GIMG, :])
                for q in range(GIMG // OCT):
                    o0 = g0 + q * OCT
                    oct_no = g * (GIMG // OCT) + q
                    p1 = ps1.tile([S, OCT, S], f32)
                    for j in range(OCT):
                        nc.tensor.matmul(
                            p1[:, j, :],
                            in_t[:, q * OCT + j, :],
                            lt_s[:],
                        )
                    p1c = mid.tile([S, OCT, S], f16)
                    copy(p1c[:], p1[:])
                    p2 = ps2.tile([S, OCT, S], f32)
                    # ((L V) R)^T, R stationary; N=512 per matmul (one PSUM
                    # bank each)
                    nc.tensor.matmul(p2[:, 0 : OCT // 2, :], r_s[:], p1c[:, 0 : OCT // 2, :])
                    nc.tensor.matmul(p2[:, OCT // 2 : OCT, :], r_s[:], p1c[:, OCT // 2 : OCT, :])
                    ot = outp.tile([S, OCT, S], f16)
                    copy(ot[:], p2[:])
                    # outputs leave per-oct on the gpsimd (SWDGE) queue: a
                    # second DMA ring, so input reads and output writes
                    # overlap on the wire and outputs drain continuously.
                    # The last few ride the (by now idle) sync HWDGE ring so
                    # the final descriptor generation isn't serialized on Q7.
                    if oct_no >= 2 * NGRP - 6 and oct_no % 2 == 0:
                        nc.sync.dma_start(out=u_out[:, o0 : o0 + OCT, :], in_=ot[:])
                    else:
                        nc.gpsimd.dma_start(out=u_out[:, o0 : o0 + OCT, :], in_=ot[:])

    nc.finalize()
    _NC_CACHE["nc"] = nc
    return nc


# ---------------------------------------------------------------- entrypoint
def _prepare_in_maps(inputs):
    """Host-side prep: L/R matrices, fp16 cast, per-core [h, b, w] packing."""
    u = np.asarray(inputs["u"], dtype=np.float32)
    assert u.shape == (BATCH, 1, S, S)
    L, R = _build_lr(
        np.asarray(inputs["alpha_base_x"], dtype=np.float64),
        np.asarray(inputs["alpha_time_coeff_x"], dtype=np.float64),
        np.asarray(inputs["alpha_time_quad_x"], dtype=np.float64),
        np.asarray(inputs["beta_base_y"], dtype=np.float64),
        np.asarray(inputs["beta_time_coeff_y"], dtype=np.float64),
        np.asarray(inputs["beta_time_quad_y"], dtype=np.float64),
    )
    lt16 = np.ascontiguousarray(L.T.astype(np.float16))
    r16 = np.ascontiguousarray(R.astype(np.float16))
    u16 = u[:, 0].astype(np.float16)  # (BATCH, S, S)
    in_maps = [
        {
            "u": np.ascontiguousarray(
                u16[c * PER_CORE : (c + 1) * PER_CORE].transpose(1, 0, 2)
            ),
            "lt": lt16,
            "rm": r16,
        }
        for c in range(N_CORES)
    ]
    return in_maps


def _gather_out(results):
    out = np.empty((BATCH, 1, S, S), dtype=np.float32)
    for c, r in enumerate(results):
        # device result is [w', b, h'] -> [b, h, w]
        out[c * PER_CORE : (c + 1) * PER_CORE, 0] = (
            r["out"].transpose(1, 2, 0).astype(np.float32)
        )
    return out


def kernel(**inputs) -> np.ndarray:
    in_maps = _prepare_in_maps(inputs)
    nc = _build_nc()
    res = run_bass_kernel_spmd(nc, in_maps, list(range(N_CORES)))
    return _gather_out(res.results)


if __name__ == "__main__":
    rng = np.random.default_rng(0)
    fake = {
        "u": rng.standard_normal((BATCH, 1, S, S), dtype=np.float32),
        "alpha_base_x": np.full(S, 2.0, np.float32),
        "alpha_base_y": np.full(S, 2.0, np.float32),
        "beta_base_x": np.full(S, 2.0, np.float32),
        "beta_base_y": np.full(S, 2.0, np.float32),
        "alpha_time_coeff_x": 0.01 * rng.standard_normal(S).astype(np.float32),
        "alpha_time_coeff_y": 0.01 * rng.standard_normal(S).astype(np.float32),
        "beta_time_coeff_x": 0.01 * rng.standard_normal(S).astype(np.float32),
        "beta_time_coeff_y": 0.01 * rng.standard_normal(S).astype(np.float32),
        "alpha_time_quad_x": 0.01 * rng.standard_normal(S).astype(np.float32),
        "alpha_time_quad_y": 0.01 * rng.standard_normal(S).astype(np.float32),
        "beta_time_quad_x": 0.01 * rng.standard_normal(S).astype(np.float32),
        "beta_time_quad_y": 0.01 * rng.standard_normal(S).astype(np.float32),
    }
    out = kernel(**fake)
    print("kernel output:", out.shape, out.dtype)
